# revision 80
# baseline (speedup 1.0000x reference)
"""Causal multi-head attention (B=4, S=2048, D=1024, H=16, RoPE) on 8 trn2 cores.

Sharding: core c -> batch c//2, head-half c%2 (8 heads / 512 dims per core).
Each core computes QKV projections for its head slice, RoPE, causal flash
attention, and a partial output projection with its Wo column slice; the host
sums the two partials per batch (the tensor-parallel all-reduce) and
transposes back.

v2 layout/scheduling (vs the 406us baseline; timeline-sim 254us):
  - pipeline warm-up: all of qb0's and half of qb1-m0's S+exp units (20)
    run interleaved with the V pass on spare phase-A PSUM banks (V-pass
    evacuations ride DVE so ACT carries only the warm exps), with enough
    pt2 buffers that no warm tile recycles before its phase-B PV; the
    warm emission finishes early so the phase-B pools' PSUM banks are
    released before the first ring allocations need them
  - bf16 everywhere outside PSUM accumulation (inputs host-cast): same PE
    rate as fp32r but no 256-wide fp32r floor, half the DMA bytes, and 2x
    DVE throughput on the elementwise ops
  - x loaded ONCE into SBUF (32KB/partition) and reused for the V pass;
    startup DMAs split across the SP+ACT HWDGE queues in first-use order
  - pass 1 is tci-outer (matches x chunk arrival); RoPE runs per (tensor,
    j, half-row) as soon as both its chunks exist (swap via 4 partition-
    block DMAs + 3 DVE bf16 TTs), spread across pass 1; pass-1/V PSUM
    evacuations all on ACT (idle in phase A)
  - causal mask is a multiplicative 0/1 bf16 TT on the diagonal 128-block
    of pt2 AFTER the exp (DVE), so S needs one matmul per (e, kt) and PE
    carries no mask matmuls at all
  - flat software pipeline over all (qb, m, kt) units: PV matmuls lag the
    S/exp pair by two units, crossing block boundaries, so PE never waits
    on the exp latency; ACT carries exps only
  - softmax normalizer: ones-column fused in V' gives the row sum; DVE
    reciprocal (psum row -> partition-0 bf16) -> rscr DRAM round trip
    (out on Pool SWDGE, one merged 2x64-partition broadcast back on SP);
    the HOP scale-multiply is deferred one m-block; pv evac is two direct
    DVE copies (DVE handles the 0->64 partition shift for e1)
  - phase C (output projection) runs as paired column groups sharing one
    store DMA, interleaved one block AND one m-slot behind the attention
    so neither the HOP scale nor the PSUM evac ever stalls PE; PSUM is
    psS 2x2 banks + a shared 4-slot ring for pv pairs and phase-C tiles
  - last block's normalizer broadcast is done with PE matmuls (ones
    outer product) instead of the DRAM round trip to shorten the tail
"""

import numpy as np

import concourse.bass as bass
import concourse.bacc as bacc
import concourse.mybir as mybir
import concourse.tile as tile
from concourse.bass import ds, ts
from concourse.bass_utils import run_bass_kernel_spmd

F32 = mybir.dt.float32
BF16 = mybir.dt.bfloat16

B, S, D, H, DK = 4, 2048, 1024, 16, 64
THETA = 10000.0
NH = 8  # heads per core
HD = NH * DK  # 512 head dims per core
P = 128
NEG = -1.0e9
EXPF = mybir.ActivationFunctionType.Exp


def build_attention_nc(nrep=1):
    nc = bacc.Bacc("TRN2", target_bir_lowering=False, debug=False)

    xT = nc.dram_tensor("xT", [D, S], BF16, kind="ExternalInput")
    wqT = nc.dram_tensor("wqT", [D, HD], BF16, kind="ExternalInput")
    wkT = nc.dram_tensor("wkT", [D, HD], BF16, kind="ExternalInput")
    wvT = nc.dram_tensor("wvT", [D, HD], BF16, kind="ExternalInput")
    woT = nc.dram_tensor("woT", [HD, D], BF16, kind="ExternalInput")
    cosP = nc.dram_tensor("cosP", [P, S], BF16, kind="ExternalInput")
    sinP = nc.dram_tensor("sinP", [P, S], BF16, kind="ExternalInput")
    trimask = nc.dram_tensor("trimask", [P, 2 * P], BF16, kind="ExternalInput")
    onesc = nc.dram_tensor("onesc", [P, P], BF16, kind="ExternalInput")
    outT = nc.dram_tensor("outT", [D, S], BF16, kind="ExternalOutput")
    rscr = nc.dram_tensor("rscr", [NH * 4, 512], BF16, kind="Internal")

    with tile.TileContext(nc) as tc:
        if nrep == 1:
            _attention_tile(
                tc, xT, wqT, wkT, wvT, woT, cosP, sinP, trimask, onesc,
                outT, rscr,
            )
        else:
            with tc.For_i(0, nrep, 1):
                _attention_tile(
                    tc, xT, wqT, wkT, wvT, woT, cosP, sinP, trimask,
                    onesc, outT, rscr,
                )
    nc.compile()
    return nc


def _attention_tile(tc, xT, wqT, wkT, wvT, woT, cosP, sinP, trimask, onesc, outT, rscr):
    nc = tc.nc

    with tc.tile_pool(name="qkv", bufs=1) as qkv:
        # ---- persistent tiles ----
        x_sb = qkv.tile([P, 8, S], BF16, tag="x")     # [k%128, k//128, t]
        QT = qkv.tile([P, 4, S], BF16, tag="QT")      # [d'%128, d'//128, t]
        KT = qkv.tile([P, 4, S], BF16, tag="KT")
        VP = qkv.tile([P, 16, 528], BF16, tag="VP")   # [t%128, t//128, 8*(64+ones+pad)]
        HOP = qkv.tile([P, 4, S], BF16, tag="HOP")    # head pairs x [128 dv, t]
        cos_sb = qkv.tile([P, S], BF16, tag="cos")
        sin_sb = qkv.tile([P, S], BF16, tag="sin")
        wq_sb = qkv.tile([P, 8, HD], BF16, tag="wq")
        wk_sb = qkv.tile([P, 8, HD], BF16, tag="wk")
        wv_sb = qkv.tile([P, 8, HD], BF16, tag="wv")
        wo_sb = qkv.tile([P, 4, D], BF16, tag="wo")
        tri_sb = qkv.tile([P, 2, P], BF16, tag="tri")

        ones_sb = qkv.tile([1, P], BF16, tag="ones1")

        # ---- bulk input DMAs, all up front on the SP HWDGE queue, in
        # first-use order (x chunk 0 and wq gate the first matmul) ----
        xT_t = xT.ap().rearrange("(o p) t -> p o t", p=P)
        wq_src = wqT.ap().rearrange("(o p) d -> p o d", p=P)
        # first mm group needs x chunk 0 + wq; split both so the halves land
        # (and the first 4-step accumulation starts) as early as possible
        # wq pieces ride the ACT HWDGE queue, x pieces the SP queue, so the
        # two issue pipelines overlap and the first 2-block accumulation can
        # start as soon as its operands land
        nc.sync.dma_start(wq_sb[:, 0:2, :], wq_src[:, 0:2, :])
        nc.scalar.dma_start(x_sb[:, 0:2, ds(0, 512)], xT_t[:, 0:2, ds(0, 512)])
        nc.sync.dma_start(wq_sb[:, 2:4, :], wq_src[:, 2:4, :])
        nc.scalar.dma_start(x_sb[:, 2:4, ds(0, 512)], xT_t[:, 2:4, ds(0, 512)])
        nc.sync.dma_start(wq_sb[:, 4:8, :], wq_src[:, 4:8, :])
        nc.scalar.dma_start(x_sb[:, 4:8, ds(0, 512)], xT_t[:, 4:8, ds(0, 512)])
        nc.scalar.dma_start(wk_sb, wkT.ap().rearrange("(o p) d -> p o d", p=P))
        for tci in range(1, 4):
            nc.sync.dma_start(x_sb[:, :, ds(tci * 512, 512)], xT_t[:, :, ds(tci * 512, 512)])
        nc.sync.dma_start(cos_sb, cosP.ap())
        nc.sync.dma_start(sin_sb, sinP.ap())
        nc.sync.dma_start(tri_sb, trimask.ap().rearrange("p (e q) -> p e q", e=2))
        nc.sync.dma_start(ones_sb, onesc.ap()[0:1, :])
        # ones columns of V' (col 64 of each 66-wide head group)
        vp_g = VP[:, :, :].rearrange("p k (h c) -> p k h c", c=66)
        nc.sync.dma_start(
            vp_g[:, :, :, 64:65],
            onesc.ap().rearrange("p (k h one) -> p k h one", k=16, one=1),
        )
        nc.sync.dma_start(wo_sb, woT.ap().rearrange("(m p) o -> p m o", p=P))
        nc.sync.dma_start(wv_sb, wvT.ap().rearrange("(o p) d -> p o d", p=P))

        # pt2 tiles live across phase A (pipeline warm-up) and phase B
        ptile_cm = tc.tile_pool(name="ptile", bufs=23)
        ptile = ptile_cm.__enter__()

        # ---- phase A: Q+K (tci-outer matches x arrival; RoPE per (dst,j)
        # once its last chunk lands), then V ----
        with (
            tc.tile_pool(name="swpool", bufs=2) as swpool,
            tc.tile_pool(name="psA", bufs=4, space="PSUM") as psA,
            tc.tile_pool(name="psW", bufs=2, space="PSUM") as psW,
        ):
            for tci in range(4):
                for dst, w_sb in ((QT, wq_sb), (KT, wk_sb)):
                    for j in range(4):
                        ps = psA.tile([P, 512], F32, tag="psA")
                        for i in range(8):
                            nc.tensor.matmul(
                                ps,
                                lhsT=w_sb[:, i, ts(j, P)],
                                rhs=x_sb[:, i, ds(tci * 512, 512)],
                                start=(i == 0),
                                stop=(i == 7),
                            )
                        nc.scalar.copy(dst[:, j, ds(tci * 512, 512)], ps)
                        if tci % 2 == 1:
                            # RoPE for this (tensor, j) half-row as soon as
                            # both its chunks exist: pair-swap via 4
                            # partition-block DMAs (Q on the SP queue, K on
                            # the ACT queue so neither backs up), 3 DVE bf16
                            # TTs. Half-row granularity spreads the DVE work
                            # across pass 1 instead of piling it at the end.
                            hsl = ds((tci // 2) * 1024, 1024)
                            qsw = swpool.tile([P, 1024], BF16, tag="qsw")
                            for blk in range(4):
                                sb = blk + (1 if blk % 2 == 0 else -1)
                                nc.sync.dma_start(
                                    qsw[blk * 32 : blk * 32 + 32, :],
                                    dst[sb * 32 : sb * 32 + 32, j, hsl],
                                )
                            tmp = swpool.tile([P, 1024], BF16, tag="rtmp")
                            nc.vector.tensor_mul(tmp, cos_sb[:, hsl], dst[:, j, hsl])
                            nc.vector.tensor_mul(qsw, sin_sb[:, hsl], qsw)
                            nc.vector.tensor_add(dst[:, j, hsl], tmp, qsw)

            # V pass (x already resident), with the first attention
            # block's S+exp units woven in on spare PSUM banks: the exp
            # chain on ACT is warmed up before phase B even starts
            warm = []

            def warm_unit(wqb, wm, kt):
                roff = kt - 4 * wqb
                c0 = max(0, 128 * roff)
                s2 = psW.tile([P, 2, 512], F32, tag="s2w")
                for e in range(2):
                    rb = e * 64
                    nc.tensor.matmul(
                        s2[:, e, c0:],
                        lhsT=KT[rb : rb + 64, wm, ts(kt, P)],
                        rhs=QT[rb : rb + 64, wm, ds(wqb * 512 + c0, 512 - c0)],
                        start=True,
                        stop=True,
                    )
                pt2 = ptile.tile([P, 2, 512], BF16, tag="pt")
                nc.scalar.activation(pt2[:, :, c0:], s2[:, :, c0:], EXPF, scale=0.125)
                if roff >= 0:
                    nc.vector.tensor_mul(
                        pt2[:, :, ds(c0, P)], pt2[:, :, ds(c0, P)], tri_sb
                    )
                warm.append((wqb, wm, kt, pt2, c0))

            warm_specs = [(0, m, kt) for m in range(4) for kt in range(4)]
            warm_specs += [(1, 0, kt) for kt in range(4)]

            for tci in range(4):
                for tt in range(4):
                    ps = psA.tile([P, 512], F32, tag="psA")
                    for i in range(8):
                        nc.tensor.matmul(
                            ps,
                            lhsT=x_sb[:, i, ds(tci * 512 + tt * P, P)],
                            rhs=wv_sb[:, i, :],
                            start=(i == 0),
                            stop=(i == 7),
                        )
                    kt_idx = tci * 4 + tt
                    # DVE, not ACT: the warm-up exps must not queue behind
                    # these evacs in ACT's in-order stream
                    nc.vector.tensor_copy(
                        vp_g[:, kt_idx, :, 0:64],
                        ps.rearrange("p (h c) -> p h c", c=64),
                    )
                    due = min(len(warm_specs), len(warm_specs) * (kt_idx + 1) // 14)
                    while len(warm) < due:
                        warm_unit(*warm_specs[len(warm)])

        # ---- phase B: attention per (qb, m) + interleaved phase C ----
        with (
            tc.tile_pool(name="srowp", bufs=4) as srowp,
            tc.tile_pool(name="scap", bufs=2) as scap,
            tc.tile_pool(name="obpool", bufs=3) as obpool,
            tc.tile_pool(name="psS", bufs=2, space="PSUM") as psS,
            tc.tile_pool(name="psPV", bufs=4, space="PSUM") as psPV,
        ):
            psC = psPV  # phase-C groups share the 4-slot ring
            outT_ap = outT.ap()
            pending_scale = []  # deferred HOP scale-mul: (m, qsl, sca)

            def flush_scale():
                while pending_scale:
                    pm, pqsl, psca = pending_scale.pop(0)
                    nc.vector.tensor_mul(HOP[:, pm, pqsl], HOP[:, pm, pqsl], psca)

            outT_t = outT_ap.rearrange("(o p) t -> p o t", p=P)

            def c_pair(cqb, ot0, tail=False):
                """Two output-projection column groups sharing one ob tile
                and one paired outT store (halves the store DMA count)."""
                ob2 = obpool.tile([P, 2, 512], BF16, tag="ob")
                for k in range(2):
                    ot = ot0 + k
                    ps = psC.tile([P, 512], F32, tag="pv", name=f"c{cqb}_{ot}")
                    for mm in range(4):
                        nc.tensor.matmul(
                            ps,
                            lhsT=wo_sb[:, mm, ts(ot, P)],
                            rhs=HOP[:, mm, ds(cqb * 512, 512)],
                            start=(mm == 0),
                            stop=(mm == 3),
                        )
                    # at the tail split evacs over ACT+DVE so the final
                    # drain pipelines
                    if tail and k:
                        nc.scalar.copy(ob2[:, k, :], ps)
                    else:
                        nc.vector.tensor_copy(ob2[:, k, :], ps)
                dma_eng = nc.sync if tail and (ot0 // 2) % 2 else nc.gpsimd
                dma_eng.dma_start(
                    outT_t[:, ot0 : ot0 + 2, ds(cqb * 512, 512)], ob2
                )

            def s_exp_unit(qb, m, kt):
                """S matmuls + exp for one (qb, m, kt) unit; returns PV args."""
                roff = kt - 4 * qb
                c0 = max(0, 128 * roff)
                qsl = ds(qb * 512, 512)
                s2 = psS.tile([P, 2, 512], F32, tag="s")
                for e in range(2):
                    rb = e * 64
                    nc.tensor.matmul(
                        s2[:, e, c0:],
                        lhsT=KT[rb : rb + 64, m, ts(kt, P)],
                        rhs=QT[rb : rb + 64, m, ds(qb * 512 + c0, 512 - c0)],
                        start=True,
                        stop=True,
                    )
                pt2 = ptile.tile([P, 2, 512], BF16, tag="pt")
                nc.scalar.activation(pt2[:, :, c0:], s2[:, :, c0:], EXPF, scale=0.125)
                if roff >= 0:
                    # causal mask: zero the upper triangle of the diagonal
                    # 128-block multiplicatively post-exp. GPSIMD, not DVE:
                    # the DVE queue carries the norm chains, whose head-of-
                    # line waits would delay the mask and stall the PV
                    nc.vector.tensor_mul(
                        pt2[:, :, ds(c0, P)], pt2[:, :, ds(c0, P)], tri_sb
                    )
                return pt2, c0

            blk_pvs = {}  # (qb, m) -> pv tile pair

            def pv_unit(qb, m, kt, pt2, c0):
                nkt = 4 * qb + 4
                if kt == 0:
                    blk_pvs[(qb, m)] = [
                        psPV.tile([P, 512], F32, tag="pv", name=f"pv{qb}{m}{e}")
                        for e in range(2)
                    ]
                pvs = blk_pvs[(qb, m)]
                for e in range(2):
                    nc.tensor.matmul(
                        pvs[e][0:65, c0:],
                        lhsT=VP[:, kt, ds((2 * m + e) * 66, 65)],
                        rhs=pt2[:, e, c0:],
                        start=(kt == 0),
                        stop=(kt == nkt - 1),
                    )

            def norm_block(qb, m, tail=False):
                """Normalizer + pv evac for a finished (qb, m) block, then the
                scheduled phase-C pair of qb-1. DVE order: deferred scale
                first, then recips + HOP copies (these free the pv ring slots
                the next block is about to claim), C evacs last."""
                qsl = ds(qb * 512, 512)
                pvs = blk_pvs.pop((qb, m))
                flush_scale()
                sca = scap.tile([P, 512], BF16, tag="sca")
                srows = []
                for e in range(2):
                    srow = srowp.tile([1, 512], BF16, tag="srow1")
                    with nc.allow_low_precision(reason="softmax normalizer to bf16"):
                        nc.vector.reciprocal(srow, pvs[e][64:65, :])
                    srows.append(srow)
                nc.vector.tensor_copy(HOP[0:64, m, qsl], pvs[0][0:64, :])
                nc.vector.tensor_copy(HOP[64:128, m, qsl], pvs[1][0:64, :])
                if not tail:
                    slot = 8 * qb + 2 * m
                    for e in range(2):
                        nc.gpsimd.dma_start(
                            rscr.ap()[slot + e : slot + e + 1, :], srows[e]
                        )
                    rsrc = bass.AP(
                        tensor=rscr.ap().tensor,
                        offset=slot * 512,
                        ap=[[512, 2], [0, 64], [1, 512]],
                    )
                    nc.sync.dma_start(sca, rsrc)
                    pending_scale.append((m, qsl, sca))
                if qb >= 1 and m >= 1:
                    c_pair(qb - 1, 2 * (m - 1), tail=tail)
                elif qb >= 2 and m == 0:
                    c_pair(qb - 2, 6, tail=tail)
                if tail:
                    # latency-critical last block: broadcast the recip rows
                    # with PE matmuls instead of the DRAM round trip (the
                    # c_pair above keeps PE busy during the recip latency)
                    bc = psS.tile([P, 2, 512], F32, tag="s", name="bcast")
                    for e in range(2):
                        nc.tensor.matmul(
                            bc[e * 64 : e * 64 + 64, 0, :],
                            lhsT=ones_sb[0:1, 0:64],
                            rhs=srows[e],
                            start=True,
                            stop=True,
                        )
                    nc.vector.tensor_copy(sca, bc[:, 0, :])
                    nc.vector.tensor_mul(HOP[:, m, qsl], HOP[:, m, qsl], sca)

            # flat software pipeline over all (qb, m, kt) units: PV matmuls
            # lag the S/exp pair by two units so PE never waits on exp latency
            units = [
                (qb, m, kt)
                for qb in range(4)
                for m in range(4)
                for kt in range(4 * qb + 4)
            ][20:]  # qb0 and half of (1, m0) warmed up during the V pass

            pipe = list(warm)

            def drain_one():
                q_, m_, k_, p_, c_ = pipe.pop(0)
                pv_unit(q_, m_, k_, p_, c_)
                if k_ == 4 * q_ + 3:
                    norm_block(q_, m_, tail=(q_ == 3 and m_ == 3))

            while len(pipe) > 2:
                drain_one()
            for qb, m, kt in units:
                pt2, c0 = s_exp_unit(qb, m, kt)
                pipe.append((qb, m, kt, pt2, c0))
                if len(pipe) > 2:
                    drain_one()
            while pipe:
                drain_one()

            # tail: the carried pair plus last qb's phase C
            c_pair(2, 6, tail=True)
            for ot0 in (0, 2, 4, 6):
                c_pair(3, ot0, tail=True)

        ptile_cm.__exit__(None, None, None)


# ---------------- host side ----------------

def _host_tables():
    import ml_dtypes

    i = np.arange(32, dtype=np.float32)
    inv_freq = (THETA ** (2.0 * i / DK)).astype(np.float32)
    t = np.arange(S, dtype=np.float32)
    ang = t[:, None] / inv_freq[None, :]  # [S, 32]
    c = np.cos(ang).astype(np.float32).T  # [32, S]
    sn = np.sin(ang).astype(np.float32).T
    cosP = np.tile(c, (4, 1))  # [128, S]
    sinP = np.tile(sn, (4, 1))
    sign = np.repeat(np.array([-1.0, 1.0, -1.0, 1.0], dtype=np.float32), 32)
    sinP = sinP * sign[:, None]

    kk = np.arange(P)[:, None]
    qq = np.arange(P)[None, :]
    keep = (kk <= qq).astype(ml_dtypes.bfloat16)  # [128,128]
    trimask = np.tile(keep, (1, 2))  # [128, 2*128] (both head halves)
    bf = ml_dtypes.bfloat16
    return cosP.astype(bf), sinP.astype(bf), trimask


_PERM = np.concatenate(
    [np.concatenate([h * 64 + np.arange(0, 64, 2), h * 64 + np.arange(1, 64, 2)])
     for h in range(NH)]
)

_NC_CACHE = {}


def make_in_maps(x, Wq, Wk, Wv, Wo):
    import ml_dtypes

    bf = ml_dtypes.bfloat16
    cosP, sinP, trimask = _host_tables()
    in_maps = []
    for c in range(8):
        b, hh = c // 2, c % 2
        sl = slice(hh * HD, (hh + 1) * HD)
        in_maps.append(
            {
                "xT": np.ascontiguousarray(x[b].T).astype(bf),
                "wqT": np.ascontiguousarray(Wq[sl, :][_PERM].T).astype(bf),
                "wkT": np.ascontiguousarray(Wk[sl, :][_PERM].T).astype(bf),
                "wvT": np.ascontiguousarray(Wv[sl, :].T).astype(bf),
                "woT": np.ascontiguousarray(Wo[:, sl].T).astype(bf),
                "cosP": cosP,
                "sinP": sinP,
                "trimask": trimask,
                "onesc": np.ones((P, P), dtype=bf),
            }
        )
    return in_maps


def gather_out(core_outs):
    out = np.empty((B, S, D), dtype=np.float32)
    for b in range(B):
        a = np.asarray(core_outs[2 * b]["outT"], dtype=np.float32)
        bb = np.asarray(core_outs[2 * b + 1]["outT"], dtype=np.float32)
        out[b] = (a + bb).T
    return out


def kernel(x, Wq, Wk, Wv, Wo):
    x = np.asarray(x, dtype=np.float32)
    Wq = np.asarray(Wq, dtype=np.float32)
    Wk = np.asarray(Wk, dtype=np.float32)
    Wv = np.asarray(Wv, dtype=np.float32)
    Wo = np.asarray(Wo, dtype=np.float32)

    if "nc" not in _NC_CACHE:
        _NC_CACHE["nc"] = build_attention_nc()
    nc = _NC_CACHE["nc"]

    in_maps = make_in_maps(x, Wq, Wk, Wv, Wo)
    res = run_bass_kernel_spmd(nc, in_maps, core_ids=list(range(8)))
    return gather_out(res.results)


# revision 83
# speedup vs baseline: 1.0017x; 1.0017x over previous
"""Causal multi-head attention (B=4, S=2048, D=1024, H=16, RoPE) on 8 trn2 cores.

Sharding: core c -> batch c//2, head-half c%2 (8 heads / 512 dims per core).
Each core computes QKV projections for its head slice, RoPE, causal flash
attention, and a partial output projection with its Wo column slice; the host
sums the two partials per batch (the tensor-parallel all-reduce) and
transposes back.

v2 layout/scheduling (vs the 406us baseline; timeline-sim 254us):
  - pipeline warm-up: all of qb0's and half of qb1-m0's S+exp units (20)
    run interleaved with the V pass on spare phase-A PSUM banks (V-pass
    evacuations ride DVE so ACT carries only the warm exps), with enough
    pt2 buffers that no warm tile recycles before its phase-B PV; the
    warm emission finishes early so the phase-B pools' PSUM banks are
    released before the first ring allocations need them
  - bf16 everywhere outside PSUM accumulation (inputs host-cast): same PE
    rate as fp32r but no 256-wide fp32r floor, half the DMA bytes, and 2x
    DVE throughput on the elementwise ops
  - x loaded ONCE into SBUF (32KB/partition) and reused for the V pass;
    startup DMAs split across the SP+ACT HWDGE queues in first-use order
  - pass 1 is tci-outer (matches x chunk arrival); RoPE runs per (tensor,
    j, half-row) as soon as both its chunks exist (swap via 4 partition-
    block DMAs + 3 DVE bf16 TTs), spread across pass 1; pass-1/V PSUM
    evacuations all on ACT (idle in phase A)
  - causal mask is a multiplicative 0/1 bf16 TT on the diagonal 128-block
    of pt2 AFTER the exp (DVE), so S needs one matmul per (e, kt) and PE
    carries no mask matmuls at all
  - flat software pipeline over all (qb, m, kt) units: PV matmuls lag the
    S/exp pair by two units, crossing block boundaries, so PE never waits
    on the exp latency; ACT carries exps only
  - softmax normalizer: ones-column fused in V' gives the row sum; DVE
    reciprocal (psum row -> partition-0 bf16) -> rscr DRAM round trip
    (out on Pool SWDGE, one merged 2x64-partition broadcast back on SP);
    the HOP scale-multiply is deferred one m-block; pv evac is two direct
    DVE copies (DVE handles the 0->64 partition shift for e1)
  - phase C (output projection) runs as paired column groups sharing one
    store DMA, interleaved one block AND one m-slot behind the attention
    so neither the HOP scale nor the PSUM evac ever stalls PE; PSUM is
    psS 2x2 banks + a shared 4-slot ring for pv pairs and phase-C tiles
  - last block's normalizer broadcast is done with PE matmuls (ones
    outer product) instead of the DRAM round trip to shorten the tail
"""

import numpy as np

import concourse.bass as bass
import concourse.bacc as bacc
import concourse.mybir as mybir
import concourse.tile as tile
from concourse.bass import ds, ts
from concourse.bass_utils import run_bass_kernel_spmd

F32 = mybir.dt.float32
BF16 = mybir.dt.bfloat16

B, S, D, H, DK = 4, 2048, 1024, 16, 64
THETA = 10000.0
NH = 8  # heads per core
HD = NH * DK  # 512 head dims per core
P = 128
NEG = -1.0e9
EXPF = mybir.ActivationFunctionType.Exp


def build_attention_nc(nrep=1):
    nc = bacc.Bacc("TRN2", target_bir_lowering=False, debug=False)

    xT = nc.dram_tensor("xT", [D, S], BF16, kind="ExternalInput")
    wqT = nc.dram_tensor("wqT", [D, HD], BF16, kind="ExternalInput")
    wkT = nc.dram_tensor("wkT", [D, HD], BF16, kind="ExternalInput")
    wvT = nc.dram_tensor("wvT", [D, HD], BF16, kind="ExternalInput")
    woT = nc.dram_tensor("woT", [HD, D], BF16, kind="ExternalInput")
    cosP = nc.dram_tensor("cosP", [P, S], BF16, kind="ExternalInput")
    sinP = nc.dram_tensor("sinP", [P, S], BF16, kind="ExternalInput")
    trimask = nc.dram_tensor("trimask", [P, 2 * P], BF16, kind="ExternalInput")
    onesc = nc.dram_tensor("onesc", [P, P], BF16, kind="ExternalInput")
    outT = nc.dram_tensor("outT", [D, S], BF16, kind="ExternalOutput")
    rscr = nc.dram_tensor("rscr", [NH * 4, 512], BF16, kind="Internal")

    with tile.TileContext(nc) as tc:
        if nrep == 1:
            _attention_tile(
                tc, xT, wqT, wkT, wvT, woT, cosP, sinP, trimask, onesc,
                outT, rscr,
            )
        else:
            with tc.For_i(0, nrep, 1):
                _attention_tile(
                    tc, xT, wqT, wkT, wvT, woT, cosP, sinP, trimask,
                    onesc, outT, rscr,
                )
    nc.compile()
    return nc


def _attention_tile(tc, xT, wqT, wkT, wvT, woT, cosP, sinP, trimask, onesc, outT, rscr):
    nc = tc.nc

    with tc.tile_pool(name="qkv", bufs=1) as qkv:
        # ---- persistent tiles ----
        x_sb = qkv.tile([P, 8, S], BF16, tag="x")     # [k%128, k//128, t]
        QT = qkv.tile([P, 4, S], BF16, tag="QT")      # [d'%128, d'//128, t]
        KT = qkv.tile([P, 4, S], BF16, tag="KT")
        VP = qkv.tile([P, 16, 528], BF16, tag="VP")   # [t%128, t//128, 8*(64+ones+pad)]
        HOP = qkv.tile([P, 4, S], BF16, tag="HOP")    # head pairs x [128 dv, t]
        cos_sb = qkv.tile([P, S], BF16, tag="cos")
        sin_sb = qkv.tile([P, S], BF16, tag="sin")
        wq_sb = qkv.tile([P, 8, HD], BF16, tag="wq")
        wk_sb = qkv.tile([P, 8, HD], BF16, tag="wk")
        wv_sb = qkv.tile([P, 8, HD], BF16, tag="wv")
        wo_sb = qkv.tile([P, 4, D], BF16, tag="wo")
        tri_sb = qkv.tile([P, 2, P], BF16, tag="tri")

        ones_sb = qkv.tile([1, P], BF16, tag="ones1")

        # ---- bulk input DMAs, all up front on the SP HWDGE queue, in
        # first-use order (x chunk 0 and wq gate the first matmul) ----
        xT_t = xT.ap().rearrange("(o p) t -> p o t", p=P)
        wq_src = wqT.ap().rearrange("(o p) d -> p o d", p=P)
        # first mm group needs x chunk 0 + wq; split both so the halves land
        # (and the first 4-step accumulation starts) as early as possible
        # wq pieces ride the ACT HWDGE queue, x pieces the SP queue, so the
        # two issue pipelines overlap and the first 2-block accumulation can
        # start as soon as its operands land
        nc.sync.dma_start(wq_sb[:, 0:2, :], wq_src[:, 0:2, :])
        nc.scalar.dma_start(x_sb[:, 0:2, ds(0, 512)], xT_t[:, 0:2, ds(0, 512)])
        nc.sync.dma_start(wq_sb[:, 2:4, :], wq_src[:, 2:4, :])
        nc.scalar.dma_start(x_sb[:, 2:4, ds(0, 512)], xT_t[:, 2:4, ds(0, 512)])
        nc.sync.dma_start(wq_sb[:, 4:8, :], wq_src[:, 4:8, :])
        nc.scalar.dma_start(x_sb[:, 4:8, ds(0, 512)], xT_t[:, 4:8, ds(0, 512)])
        nc.scalar.dma_start(wk_sb, wkT.ap().rearrange("(o p) d -> p o d", p=P))
        for tci in range(1, 4):
            nc.sync.dma_start(x_sb[:, :, ds(tci * 512, 512)], xT_t[:, :, ds(tci * 512, 512)])
        nc.sync.dma_start(cos_sb, cosP.ap())
        nc.sync.dma_start(sin_sb, sinP.ap())
        nc.sync.dma_start(tri_sb, trimask.ap().rearrange("p (e q) -> p e q", e=2))
        nc.sync.dma_start(ones_sb, onesc.ap()[0:1, :])
        # ones columns of V' (col 64 of each 66-wide head group)
        vp_g = VP[:, :, :].rearrange("p k (h c) -> p k h c", c=66)
        nc.sync.dma_start(
            vp_g[:, :, :, 64:65],
            onesc.ap().rearrange("p (k h one) -> p k h one", k=16, one=1),
        )
        nc.sync.dma_start(wo_sb, woT.ap().rearrange("(m p) o -> p m o", p=P))
        nc.sync.dma_start(wv_sb, wvT.ap().rearrange("(o p) d -> p o d", p=P))

        # pt2 tiles live across phase A (pipeline warm-up) and phase B
        ptile_cm = tc.tile_pool(name="ptile", bufs=23)
        ptile = ptile_cm.__enter__()

        # ---- phase A: Q+K (tci-outer matches x arrival; RoPE per (dst,j)
        # once its last chunk lands), then V ----
        with (
            tc.tile_pool(name="swpool", bufs=2) as swpool,
            tc.tile_pool(name="psA", bufs=4, space="PSUM") as psA,
            tc.tile_pool(name="psW", bufs=2, space="PSUM") as psW,
        ):
            for tci in range(4):
                for dst, w_sb in ((QT, wq_sb), (KT, wk_sb)):
                    for j in range(4):
                        ps = psA.tile([P, 512], F32, tag="psA")
                        for i in range(8):
                            nc.tensor.matmul(
                                ps,
                                lhsT=w_sb[:, i, ts(j, P)],
                                rhs=x_sb[:, i, ds(tci * 512, 512)],
                                start=(i == 0),
                                stop=(i == 7),
                            )
                        nc.scalar.copy(dst[:, j, ds(tci * 512, 512)], ps)
                        if tci % 2 == 1:
                            # RoPE for this (tensor, j) half-row as soon as
                            # both its chunks exist: pair-swap via 4
                            # partition-block DMAs (Q on the SP queue, K on
                            # the ACT queue so neither backs up), 3 DVE bf16
                            # TTs. Half-row granularity spreads the DVE work
                            # across pass 1 instead of piling it at the end.
                            hsl = ds((tci // 2) * 1024, 1024)
                            qsw = swpool.tile([P, 1024], BF16, tag="qsw")
                            for blk in range(4):
                                sb = blk + (1 if blk % 2 == 0 else -1)
                                nc.sync.dma_start(
                                    qsw[blk * 32 : blk * 32 + 32, :],
                                    dst[sb * 32 : sb * 32 + 32, j, hsl],
                                )
                            tmp = swpool.tile([P, 1024], BF16, tag="rtmp")
                            nc.vector.tensor_mul(tmp, cos_sb[:, hsl], dst[:, j, hsl])
                            nc.vector.tensor_mul(qsw, sin_sb[:, hsl], qsw)
                            nc.vector.tensor_add(dst[:, j, hsl], tmp, qsw)

            # V pass (x already resident), with the first attention
            # block's S+exp units woven in on spare PSUM banks: the exp
            # chain on ACT is warmed up before phase B even starts
            warm = []

            def warm_unit(wqb, wm, kt):
                roff = kt - 4 * wqb
                c0 = max(0, 128 * roff)
                s2 = psW.tile([P, 2, 512], F32, tag="s2w")
                for e in range(2):
                    rb = e * 64
                    nc.tensor.matmul(
                        s2[:, e, c0:],
                        lhsT=KT[rb : rb + 64, wm, ts(kt, P)],
                        rhs=QT[rb : rb + 64, wm, ds(wqb * 512 + c0, 512 - c0)],
                        start=True,
                        stop=True,
                    )
                pt2 = ptile.tile([P, 2, 512], BF16, tag="pt")
                nc.scalar.activation(pt2[:, :, c0:], s2[:, :, c0:], EXPF, scale=0.125)
                if roff >= 0:
                    nc.vector.tensor_mul(
                        pt2[:, :, ds(c0, P)], pt2[:, :, ds(c0, P)], tri_sb
                    )
                warm.append((wqb, wm, kt, pt2, c0))

            warm_specs = [(0, m, kt) for m in range(4) for kt in range(4)]
            warm_specs += [(1, 0, kt) for kt in range(4)]

            for tci in range(4):
                for tt in range(4):
                    ps = psA.tile([P, 512], F32, tag="psA")
                    for i in range(8):
                        nc.tensor.matmul(
                            ps,
                            lhsT=x_sb[:, i, ds(tci * 512 + tt * P, P)],
                            rhs=wv_sb[:, i, :],
                            start=(i == 0),
                            stop=(i == 7),
                        )
                    kt_idx = tci * 4 + tt
                    # DVE, not ACT: the warm-up exps must not queue behind
                    # these evacs in ACT's in-order stream
                    nc.vector.tensor_copy(
                        vp_g[:, kt_idx, :, 0:64],
                        ps.rearrange("p (h c) -> p h c", c=64),
                    )
                    due = min(len(warm_specs), len(warm_specs) * (kt_idx + 1) // 14)
                    while len(warm) < due:
                        warm_unit(*warm_specs[len(warm)])

        # ---- phase B: attention per (qb, m) + interleaved phase C ----
        with (
            tc.tile_pool(name="srowp", bufs=4) as srowp,
            tc.tile_pool(name="scap", bufs=2) as scap,
            tc.tile_pool(name="obpool", bufs=3) as obpool,
            tc.tile_pool(name="psS", bufs=2, space="PSUM") as psS,
            tc.tile_pool(name="psPV", bufs=4, space="PSUM") as psPV,
        ):
            psC = psPV  # phase-C groups share the 4-slot ring
            outT_ap = outT.ap()
            pending_scale = []  # deferred HOP scale-mul: (m, qsl, sca)

            def flush_scale():
                while pending_scale:
                    pm, pqsl, psca = pending_scale.pop(0)
                    nc.vector.tensor_mul(HOP[:, pm, pqsl], HOP[:, pm, pqsl], psca)

            outT_t = outT_ap.rearrange("(o p) t -> p o t", p=P)

            def c_pair(cqb, ot0, tail=False):
                """Two output-projection column groups sharing one ob tile
                and one paired outT store (halves the store DMA count)."""
                ob2 = obpool.tile([P, 2, 512], BF16, tag="ob")
                for k in range(2):
                    ot = ot0 + k
                    ps = psC.tile([P, 512], F32, tag="pv", name=f"c{cqb}_{ot}")
                    for mm in range(4):
                        nc.tensor.matmul(
                            ps,
                            lhsT=wo_sb[:, mm, ts(ot, P)],
                            rhs=HOP[:, mm, ds(cqb * 512, 512)],
                            start=(mm == 0),
                            stop=(mm == 3),
                        )
                    # at the tail split evacs over ACT+DVE so the final
                    # drain pipelines
                    if tail and k:
                        nc.scalar.copy(ob2[:, k, :], ps)
                    else:
                        nc.vector.tensor_copy(ob2[:, k, :], ps)
                if tail:
                    dma_eng = nc.sync if (ot0 // 2) % 2 else nc.scalar
                else:
                    dma_eng = nc.gpsimd
                dma_eng.dma_start(
                    outT_t[:, ot0 : ot0 + 2, ds(cqb * 512, 512)], ob2
                )

            def s_exp_unit(qb, m, kt):
                """S matmuls + exp for one (qb, m, kt) unit; returns PV args."""
                roff = kt - 4 * qb
                c0 = max(0, 128 * roff)
                qsl = ds(qb * 512, 512)
                s2 = psS.tile([P, 2, 512], F32, tag="s")
                for e in range(2):
                    rb = e * 64
                    nc.tensor.matmul(
                        s2[:, e, c0:],
                        lhsT=KT[rb : rb + 64, m, ts(kt, P)],
                        rhs=QT[rb : rb + 64, m, ds(qb * 512 + c0, 512 - c0)],
                        start=True,
                        stop=True,
                    )
                pt2 = ptile.tile([P, 2, 512], BF16, tag="pt")
                nc.scalar.activation(pt2[:, :, c0:], s2[:, :, c0:], EXPF, scale=0.125)
                if roff >= 0:
                    # causal mask: zero the upper triangle of the diagonal
                    # 128-block multiplicatively post-exp. GPSIMD, not DVE:
                    # the DVE queue carries the norm chains, whose head-of-
                    # line waits would delay the mask and stall the PV
                    nc.vector.tensor_mul(
                        pt2[:, :, ds(c0, P)], pt2[:, :, ds(c0, P)], tri_sb
                    )
                return pt2, c0

            blk_pvs = {}  # (qb, m) -> pv tile pair

            def pv_unit(qb, m, kt, pt2, c0):
                nkt = 4 * qb + 4
                if kt == 0:
                    blk_pvs[(qb, m)] = [
                        psPV.tile([P, 512], F32, tag="pv", name=f"pv{qb}{m}{e}")
                        for e in range(2)
                    ]
                pvs = blk_pvs[(qb, m)]
                for e in range(2):
                    nc.tensor.matmul(
                        pvs[e][0:65, c0:],
                        lhsT=VP[:, kt, ds((2 * m + e) * 66, 65)],
                        rhs=pt2[:, e, c0:],
                        start=(kt == 0),
                        stop=(kt == nkt - 1),
                    )

            def norm_block(qb, m, tail=False):
                """Normalizer + pv evac for a finished (qb, m) block, then the
                scheduled phase-C pair of qb-1. DVE order: deferred scale
                first, then recips + HOP copies (these free the pv ring slots
                the next block is about to claim), C evacs last."""
                qsl = ds(qb * 512, 512)
                pvs = blk_pvs.pop((qb, m))
                flush_scale()
                sca = scap.tile([P, 512], BF16, tag="sca")
                srows = []
                for e in range(2):
                    srow = srowp.tile([1, 512], BF16, tag="srow1")
                    with nc.allow_low_precision(reason="softmax normalizer to bf16"):
                        nc.vector.reciprocal(srow, pvs[e][64:65, :])
                    srows.append(srow)
                nc.vector.tensor_copy(HOP[0:64, m, qsl], pvs[0][0:64, :])
                nc.vector.tensor_copy(HOP[64:128, m, qsl], pvs[1][0:64, :])
                if not tail:
                    slot = 8 * qb + 2 * m
                    for e in range(2):
                        nc.gpsimd.dma_start(
                            rscr.ap()[slot + e : slot + e + 1, :], srows[e]
                        )
                    rsrc = bass.AP(
                        tensor=rscr.ap().tensor,
                        offset=slot * 512,
                        ap=[[512, 2], [0, 64], [1, 512]],
                    )
                    nc.sync.dma_start(sca, rsrc)
                    pending_scale.append((m, qsl, sca))
                if qb >= 1 and m >= 1:
                    c_pair(qb - 1, 2 * (m - 1), tail=tail)
                elif qb >= 2 and m == 0:
                    c_pair(qb - 2, 6, tail=tail)
                if tail:
                    # latency-critical last block: broadcast the recip rows
                    # with PE matmuls instead of the DRAM round trip (the
                    # c_pair above keeps PE busy during the recip latency)
                    bc = psS.tile([P, 2, 512], F32, tag="s", name="bcast")
                    for e in range(2):
                        nc.tensor.matmul(
                            bc[e * 64 : e * 64 + 64, 0, :],
                            lhsT=ones_sb[0:1, 0:64],
                            rhs=srows[e],
                            start=True,
                            stop=True,
                        )
                    nc.vector.tensor_copy(sca, bc[:, 0, :])
                    nc.vector.tensor_mul(HOP[:, m, qsl], HOP[:, m, qsl], sca)

            # flat software pipeline over all (qb, m, kt) units: PV matmuls
            # lag the S/exp pair by two units so PE never waits on exp latency
            units = [
                (qb, m, kt)
                for qb in range(4)
                for m in range(4)
                for kt in range(4 * qb + 4)
            ][20:]  # qb0 and half of (1, m0) warmed up during the V pass

            pipe = list(warm)

            def drain_one():
                q_, m_, k_, p_, c_ = pipe.pop(0)
                pv_unit(q_, m_, k_, p_, c_)
                if k_ == 4 * q_ + 3:
                    norm_block(q_, m_, tail=(q_ == 3 and m_ == 3))

            while len(pipe) > 2:
                drain_one()
            for qb, m, kt in units:
                pt2, c0 = s_exp_unit(qb, m, kt)
                pipe.append((qb, m, kt, pt2, c0))
                if len(pipe) > 2:
                    drain_one()
            while pipe:
                drain_one()

            # tail: the carried pair plus last qb's phase C
            c_pair(2, 6, tail=True)
            for ot0 in (0, 2, 4, 6):
                c_pair(3, ot0, tail=True)

        ptile_cm.__exit__(None, None, None)


# ---------------- host side ----------------

def _host_tables():
    import ml_dtypes

    i = np.arange(32, dtype=np.float32)
    inv_freq = (THETA ** (2.0 * i / DK)).astype(np.float32)
    t = np.arange(S, dtype=np.float32)
    ang = t[:, None] / inv_freq[None, :]  # [S, 32]
    c = np.cos(ang).astype(np.float32).T  # [32, S]
    sn = np.sin(ang).astype(np.float32).T
    cosP = np.tile(c, (4, 1))  # [128, S]
    sinP = np.tile(sn, (4, 1))
    sign = np.repeat(np.array([-1.0, 1.0, -1.0, 1.0], dtype=np.float32), 32)
    sinP = sinP * sign[:, None]

    kk = np.arange(P)[:, None]
    qq = np.arange(P)[None, :]
    keep = (kk <= qq).astype(ml_dtypes.bfloat16)  # [128,128]
    trimask = np.tile(keep, (1, 2))  # [128, 2*128] (both head halves)
    bf = ml_dtypes.bfloat16
    return cosP.astype(bf), sinP.astype(bf), trimask


_PERM = np.concatenate(
    [np.concatenate([h * 64 + np.arange(0, 64, 2), h * 64 + np.arange(1, 64, 2)])
     for h in range(NH)]
)

_NC_CACHE = {}


def make_in_maps(x, Wq, Wk, Wv, Wo):
    import ml_dtypes

    bf = ml_dtypes.bfloat16
    cosP, sinP, trimask = _host_tables()
    in_maps = []
    for c in range(8):
        b, hh = c // 2, c % 2
        sl = slice(hh * HD, (hh + 1) * HD)
        in_maps.append(
            {
                "xT": np.ascontiguousarray(x[b].T).astype(bf),
                "wqT": np.ascontiguousarray(Wq[sl, :][_PERM].T).astype(bf),
                "wkT": np.ascontiguousarray(Wk[sl, :][_PERM].T).astype(bf),
                "wvT": np.ascontiguousarray(Wv[sl, :].T).astype(bf),
                "woT": np.ascontiguousarray(Wo[:, sl].T).astype(bf),
                "cosP": cosP,
                "sinP": sinP,
                "trimask": trimask,
                "onesc": np.ones((P, P), dtype=bf),
            }
        )
    return in_maps


def gather_out(core_outs):
    out = np.empty((B, S, D), dtype=np.float32)
    for b in range(B):
        a = np.asarray(core_outs[2 * b]["outT"], dtype=np.float32)
        bb = np.asarray(core_outs[2 * b + 1]["outT"], dtype=np.float32)
        out[b] = (a + bb).T
    return out


def kernel(x, Wq, Wk, Wv, Wo):
    x = np.asarray(x, dtype=np.float32)
    Wq = np.asarray(Wq, dtype=np.float32)
    Wk = np.asarray(Wk, dtype=np.float32)
    Wv = np.asarray(Wv, dtype=np.float32)
    Wo = np.asarray(Wo, dtype=np.float32)

    if "nc" not in _NC_CACHE:
        _NC_CACHE["nc"] = build_attention_nc()
    nc = _NC_CACHE["nc"]

    in_maps = make_in_maps(x, Wq, Wk, Wv, Wo)
    res = run_bass_kernel_spmd(nc, in_maps, core_ids=list(range(8)))
    return gather_out(res.results)
